# revision 23
# baseline (speedup 1.0000x reference)
"""Trainium2 Bass kernel for nn_Fine_Change_Moment3.

Math (from the reference):
  - input (16,512,512,16) [b,y,x,t]; fc_weight3 (262144,16,6) per-patch 16x6.
  - Only channel 0 of the CAM survives (cam[:, 0]), so only
    fc_weight3[:, :, 0] matters (host-sliced).
  - Per 4x4 patch n=(b,gy,gx): cam0[t] = sum_k patch[k,t] * w[n,k]
  - v = (cam0 - min_t) / max_t(cam0 - min_t)
  - top[b,t] = v arranged (gy,gx); up = A @ top @ A^T with A the 128->512
    bilinear (align_corners) interp matrix; output (b*512*512, 16) f32.

Distribution: data-parallel over batch, 2 batches per core, 8 cores.

fp16 end-to-end on device: the host casts input and weights to fp16 and
casts the fp16 output back to f32, halving HBM traffic (the DMA roofline)
and doubling DVE/PE throughput (2e-2 tolerance; fp16 leaves ~1e-2 margin).

Engine queues are FIFO, so program order is the schedule. The two batches
are software-pipelined explicitly:

  A: load+weight+reduce (s123) b0          DVE-paced, in_b0 streams
  B: norm+transpose+M1 b0; prefetch in_b1
  C: up/out b0 INTERLEAVED with s123 b1    ACT+PE run b0's tail while
                                           DVE consumes b1's input;
                                           out_b0 + in_b1 share DMA
  D: norm+transpose+M1 b1
  E: up/out b1                             copies split DVE/ACT (nothing
                                           queues behind them)

Stage detail:
  s123: DVE multiply by per-patch weights (weights host-duplicated x2 so
        the t-broadcast keeps a dense step-1 inner pair -> 2x DVE mode),
        one DVE add pass folding px pairs into p2[(px2,gx,t)] (px2
        outermost so matmul rhs chunks stay dense), then doubled
        0/1-selection matmuls reduce py AND the final px2 pair via PSUM
        accumulation -> cam[gy][(gx,t)] fp32.
  norm: v = cam - min only (computed per cam half: the gx<64 selection
        groups stop one input unit earlier, so that half of the norm
        runs before the last unit and only the gx>=64 half sits on the
        critical chain). The 1/max scale is applied AFTER the per-t
        transposes (elementwise, commutes), so the transposes depend
        only on the subtract while max/recip overlap them; rx rides the
        transpose pass as a 17th 128x128 tile and scales topT in place.
        Emitted at high priority so the scheduler does not stretch the
        chain by interleaving b1 multiplies into it.
  M1:   PE per-t transposes then M1_t = topT_t^T @ A^T; casts into
        m1i[gy][(sx,t)] are sx-halved so the xh=0 up-units (emitted
        xh-major) wait on only half of them. For b1 the whole stage is
        emitted at high priority so its ACT ops do not queue behind
        up(0)'s remaining casts (all of E depends on m1i(1)).
  up:   PE (A^T chunk)^T @ m1i chunk -> PSUM, copies cast to fp16
        staging (split across ACT and DVE by phase load), contiguous
        DMA out (the final unit's DMA is split in half to start
        earlier). On the last batch the idle cam PSUM pool is borrowed
        as a third ring slot so the PE runs ahead of the copies.

Engine facts this schedule is built around (measured):
  - DVE ~1.04ns/elem (2x for fp16 SBUF-to-SBUF with packed inner pair);
    f32/PSUM reads are 1x. ACT ~0.9-1.1ns/elem regardless of dtype.
    gpsimd cannot access PSUM at all, so all PSUM->SBUF casts must go
    through ACT or DVE - they are the binding resource of the back half.
  - TRN2 matmul may only emit f32 to PSUM, a matmul dst cannot cross a
    2KB PSUM bank, and the PE runs ~0.83ns/col (PSTATE_MID) unless
    continuously busy for 3us+ (never happens here).
  - DMA sustains ~337GB/s; in+out+weights ~36MB/core => ~106us floor.
"""

import numpy as np

B, S, T, PP = 16, 512, 16, 4
G = S // PP          # 128 patch grid
NCORES = 8
BPC = B // NCORES    # 2 batches per core

_CACHE = {}


def _interp_matrix_np(n_in, n_out):
    # mirrors the reference's align_corners=True bilinear matrix
    coords = np.arange(n_out, dtype=np.float32) * ((n_in - 1) / (n_out - 1))
    i0 = np.clip(np.floor(coords).astype(np.int64), 0, n_in - 2)
    w = coords - i0.astype(np.float32)
    A = np.zeros((n_out, n_in), dtype=np.float32)
    rows = np.arange(n_out)
    np.add.at(A, (rows, i0), 1.0 - w)
    np.add.at(A, (rows, i0 + 1), w)
    return A  # (n_out, n_in)


def _build_program():
    from contextlib import ExitStack
    import concourse.bacc as bacc
    import concourse.tile as tile
    import concourse.mybir as mybir

    f32 = mybir.dt.float32
    f16 = mybir.dt.float16
    Alu = mybir.AluOpType
    Ax = mybir.AxisListType

    nc = bacc.Bacc("TRN2", target_bir_lowering=False, debug=False,
                   num_devices=NCORES)

    x_d = nc.dram_tensor("x", [BPC, S, S, T], f16, kind="ExternalInput")
    w_d = nc.dram_tensor("w", [BPC, 128, 4096], f16, kind="ExternalInput")
    at_d = nc.dram_tensor("at", [128, 512], f16, kind="ExternalInput")
    sel_d = nc.dram_tensor("sel", [128, 512], f16, kind="ExternalInput")
    id_d = nc.dram_tensor("ident", [128, 128], f16, kind="ExternalInput")
    y_d = nc.dram_tensor("y", [BPC, S, S, T], f16, kind="ExternalOutput")

    # input view: [b][yt][xh][y_row=128][(x256 t16)=4096]
    x_v = x_d.ap().rearrange("b (yt p) (xh xx) t -> b yt xh p (xx t)",
                             p=128, xh=2)
    # output view: [b][syc][xh][sy=128][(xx256 t16)=4096]
    y_v = y_d.ap().rearrange("b (syc sy) (xh xx) t -> b syc xh sy (xx t)",
                             syc=4, xh=2)

    with tile.TileContext(nc) as tc, ExitStack() as ctx:
        consts = ctx.enter_context(tc.tile_pool(name="consts", bufs=1))
        pin = ctx.enter_context(tc.tile_pool(name="pin", bufs=10))
        pw = ctx.enter_context(tc.tile_pool(name="pw", bufs=8))
        pp2 = ctx.enter_context(tc.tile_pool(name="pp2", bufs=4))
        pv = ctx.enter_context(tc.tile_pool(name="pv", bufs=1))
        ptop = ctx.enter_context(tc.tile_pool(name="ptop", bufs=1))
        pm1 = ctx.enter_context(tc.tile_pool(name="pm1", bufs=1))
        pst = ctx.enter_context(tc.tile_pool(name="pst", bufs=5))
        # PSUM: cam+transpose share a 4-bank pool; m1/up share a
        # 2x 2-bank pool -> static total 8 banks.
        ppsc = ctx.enter_context(tc.tile_pool(name="ppsc", bufs=1,
                                              space="PSUM"))
        ppsmu = ctx.enter_context(tc.tile_pool(name="ppsmu", bufs=2,
                                               space="PSUM"))

        at_sb = consts.tile([128, 512], f16)
        sel_sb = consts.tile([128, 512], f16)
        id_sb = consts.tile([128, 128], f16)

        state = {}

        def s123_open(b):
            # weight prefetch in per-yt chunks (first multiply only waits
            # on chunk 0); cam PSUM is allocated lazily at its first
            # matmul so the ppsc ring order stays cam(b)/tp(b)
            wv4 = w_d.ap().rearrange("b p (yt c) -> b yt p c", yt=4)
            wts = []
            for yt in range(4):
                wt = pw.tile([128, 1024], f16, tag="w", name="w_sb")
                nc.sync.dma_start(wt[:], wv4[b, yt])
                wts.append(wt)
            state[b] = {"cam_ps": None, "w_sb": wts, "p2": None}

        def s123_unit(b, u, pre=None, fold_pool=False):
            # one (yt, xh) input unit: DMA + multiply + add tree (+ cam
            # matmuls when the yt pair completes)
            yt, xh = u // 2, u % 2
            st = state[b]
            w_sb = st["w_sb"]
            if xh == 0:
                st["p2"] = pp2.tile([128, 4096], f16, tag="p2", name="p2")
            p2 = st["p2"]
            if pre is None:
                it = pin.tile([128, 4096], f16, tag="in")
                nc.sync.dma_start(it[:], x_v[b, yt, xh])
            else:
                it = pre
            itv = it[:].rearrange("p (x tp two) -> p x tp two",
                                  tp=T // 2, two=2)
            wv = (w_sb[yt][:, xh * 512:(xh + 1) * 512]
                  .rearrange("p (x two) -> p x two", two=2)
                  .unsqueeze(2)
                  .broadcast_to([128, 256, T // 2, 2]))
            nc.vector.tensor_tensor(itv, itv, wv, op=Alu.mult)
            # single add pass folds px {0,2}+{1,3} -> p2[(px2, gx, t)]
            # (px2 outermost so the doubled selection matmuls read dense
            # 512-chunks); the final px2 pair is summed by PSUM
            # accumulation in the selection matmuls
            pr = it[:].rearrange("p (gx pxp px2 t) -> p px2 gx t pxp",
                                 pxp=2, px2=2, t=T)
            rout = (p2[:].rearrange("p (px2 gx t) -> p px2 gx t",
                                    px2=2, t=T)[:, :, xh * 64:(xh + 1) * 64])
            # gpsimd folds run ~4x slower but fully concurrent with the
            # DVE mult stream; giving it the first half of b1's folds
            # shortens the DVE spine in phase C
            feng = nc.gpsimd if fold_pool else nc.vector
            feng.tensor_tensor(rout, pr[:, :, :, :, 0],
                               pr[:, :, :, :, 1], op=Alu.add)
            # selection matmuls for THIS xh-half (fc 2xh, 2xh+1): they
            # read only this unit's fold output, so the gx<64 cam half
            # finalizes at unit 6 and norm_h(b,0) can run before unit 7
            if st["cam_ps"] is None:
                st["cam_ps"] = ppsc.tile([128, 2048], f32, tag="cam",
                                     name="cam_ps")
            cam_ps = st["cam_ps"]
            for fcl in range(2):
                fc = 2 * xh + fcl
                for px2 in range(2):
                    nc.tensor.matmul(
                        cam_ps[:, fc * 512:(fc + 1) * 512],
                        lhsT=sel_sb[:, yt * 128:(yt + 1) * 128],
                        rhs=p2[:, px2 * 2048 + fc * 512:
                               px2 * 2048 + (fc + 1) * 512],
                        start=(yt == 0 and px2 == 0),
                        stop=(yt == 3 and px2 == 1),
                    )

        def norm_h(b, h):
            # v = cam - min only; the 1/max scaling is applied AFTER the
            # per-t transposes (it commutes elementwise), so the
            # transposes depend only on the subtract and the max/recip
            # run concurrently with the PE transposes. Split per cam
            # half: the selection-matmul groups for gx 0..63 stop one
            # input unit earlier than gx 64..127, so half the norm runs
            # before the last unit's mult/fold and only the h=1 half
            # sits on the critical chain.
            st = state[b]
            cam_ps = st["cam_ps"]
            if h == 0:
                st["v"] = pv.tile([128, 2048], f16, tag="v", name="v")
                st["mn"] = pv.tile([128, 128], f16, tag="mn", name="mn")
                st["mx"] = pv.tile([128, 128], f32, tag="mx", name="mx")
                st["rx"] = pv.tile([128, 128], f16, tag="rx", name="rx")
            v, mn, mx, rx = st["v"], st["mn"], st["mx"], st["rx"]
            cam3 = (cam_ps[:, h * 1024:(h + 1) * 1024]
                    .rearrange("p (gx t) -> p gx t", t=T))
            v3 = (v[:, h * 1024:(h + 1) * 1024]
                  .rearrange("p (gx t) -> p gx t", t=T))
            mnh = mn[:, h * 64:(h + 1) * 64]
            mxh = mx[:, h * 64:(h + 1) * 64]
            rxh = rx[:, h * 64:(h + 1) * 64]
            nc.vector.tensor_reduce(mnh, cam3, axis=Ax.X, op=Alu.min)
            mnb = mnh.unsqueeze(2).broadcast_to([128, 64, T])
            nc.vector.tensor_tensor(v3, cam3, mnb, op=Alu.subtract)
            nc.vector.tensor_reduce(mxh, v3, axis=Ax.X, op=Alu.max)
            with nc.allow_low_precision(reason="1/max in f16: |err|<1e-3 "
                                        "vs the 2e-2 gate; everything "
                                        "downstream is f16 anyway"):
                nc.vector.reciprocal(rxh, mxh)

        def tpm1(b, help_dve):
            st = state[b]
            v = st["v"]
            rx = st["rx"]

            # per-t 128x128 transposes -> topT[gx][(t,gy)]; these wait
            # only on the norm subtract (v is unscaled). rx rides along
            # as a 17th transpose; the scale lands on topT afterwards.
            topT = ptop.tile([128, 2048], f16, tag="top")
            rxT = ptop.tile([128, 128], f16, tag="rxT")
            vt = v[:].rearrange("p (gx t) -> p t gx", t=T)
            # 2176 wide: cols 2048+ hold the rx transpose (the tag's
            # slot is sized by the f32 cam tile anyway)
            tp_ps = ppsc.tile([128, 2176], f16, tag="cam")
            for t in range(T):
                nc.tensor.transpose(tp_ps[:, t * 128:(t + 1) * 128],
                                    vt[:, t, :], id_sb[:])
            nc.tensor.transpose(tp_ps[:, 2048:2176], rx[:], id_sb[:])
            if help_dve:
                # b1: the DVE is idle here while ACT still drains up(0)
                # casts; fp16 PSUM->SBUF on the DVE is 2x-capable
                nc.vector.tensor_copy(topT[:], tp_ps[:, 0:2048])
                nc.scalar.copy(rxT[:], tp_ps[:, 2048:2176])
            else:
                nc.scalar.copy(topT[:], tp_ps[:, 0:2048])
                nc.vector.tensor_copy(rxT[:], tp_ps[:, 2048:2176])
            # apply the 1/max scale to the transposed plane: topT[gx,t,gy]
            # *= rxT[gx,gy] (broadcast over t; gy inner stride-1 pair)
            topTv = topT[:].rearrange("p (t gy) -> p t gy", t=T)
            rxb = rxT[:].unsqueeze(1).broadcast_to([128, T, 128])
            nc.vector.tensor_tensor(topTv, topTv, rxb, op=Alu.mult)

            # M1 per t -> m1i[gy][(sx,t)] fp16, pair-interleaved
            m1i = pm1.tile([128, 8192], f16, tag="m1i")
            m1iv = m1i[:].rearrange("p (sx t) -> p sx t", t=T)
            for tq in range(8):
                # on the last batch, borrow the idle cam pool as a third
                # ring slot so the PE can run ahead of the copies
                if help_dve and tq % 3 == 2:
                    m1_ps = ppsc.tile([128, 1024], f32, tag="cam",
                                      name="m1_ps")
                else:
                    m1_ps = ppsmu.tile([128, 1024], f32, tag="mu",
                                       name="m1_ps")
                for tl in range(2):
                    t = tq * 2 + tl
                    nc.tensor.matmul(
                        m1_ps[:, tl * 512:(tl + 1) * 512],
                        lhsT=topT[:, t * 128:(t + 1) * 128],
                        rhs=at_sb[:],
                        start=True, stop=True,
                    )
                # sx-halved casts: the xh=0 up-units depend only on the
                # sx<256 halves, so they start ~half an M1-cast-round
                # earlier (up-units are emitted xh-major)
                csrc = m1_ps[:].rearrange("p (tl sx) -> p sx tl", tl=2)
                for sh in range(2):
                    csh = csrc[:, sh * 256:(sh + 1) * 256]
                    cdst = m1iv[:, sh * 256:(sh + 1) * 256,
                                tq * 2:(tq + 1) * 2]
                    if help_dve and (tq + sh) % 2 == 1:
                        nc.vector.tensor_copy(cdst, csh)
                    else:
                        nc.scalar.copy(cdst, csh)
            st["m1i"] = m1i

        up_ctr = [0]

        def up_unit(b, u, n_dve=0):
            # one (syc, xh) output unit: 4x (2 matmuls + copy), DMA out
            syc, xh = u // 2, u % 2
            m1i = state[b]["m1i"]
            stg = pst.tile([128, 4096], f16, tag="stg")
            for sxg in range(4):
                gi = up_ctr[0]
                up_ctr[0] += 1
                if b == BPC - 1 and gi % 3 == 2:
                    up_ps = ppsc.tile([128, 1024], f32, tag="cam",
                                      name="up_ps")
                else:
                    up_ps = ppsmu.tile([128, 1024], f32, tag="mu",
                                       name="up_ps")
                for sxl in range(2):
                    sxblk = (xh * 4 + sxg) * 2 + sxl
                    nc.tensor.matmul(
                        up_ps[:, sxl * 512:(sxl + 1) * 512],
                        lhsT=at_sb[:, syc * 128:(syc + 1) * 128],
                        rhs=m1i[:, sxblk * 512:(sxblk + 1) * 512],
                        start=True, stop=True,
                    )
                dst = stg[:, sxg * 1024:(sxg + 1) * 1024]
                if (n_dve >= 2 and sxg % 2 == 1) or (n_dve == 1 and sxg == 3):
                    nc.vector.tensor_copy(dst, up_ps[:])
                else:
                    nc.scalar.copy(dst, up_ps[:])
                if b == BPC - 1 and u == 7 and sxg == 1:
                    nc.sync.dma_start(y_v[b, syc, xh][:, 0:2048],
                                      stg[:, 0:2048])
            if b == BPC - 1 and u == 7:
                nc.sync.dma_start(y_v[b, syc, xh][:, 2048:4096],
                                  stg[:, 2048:4096])
            else:
                nc.sync.dma_start(y_v[b, syc, xh], stg[:])

        # ---- phase A: s123(b0); the first input tile's DMA is issued
        # before the weight chunks; consts ride behind the first DMAs
        it0 = pin.tile([128, 4096], f16, tag="in", name="it0")
        nc.sync.dma_start(it0[:, 0:2048], x_v[0, 0, 0][:, 0:2048])
        nc.sync.dma_start(it0[:, 2048:4096], x_v[0, 0, 0][:, 2048:4096])
        s123_open(0)
        # sel_sb must be loaded before unit 0 (its selection matmuls now
        # run every unit, and emission order defines the dependency)
        nc.sync.dma_start(sel_sb[:], sel_d.ap())
        s123_unit(0, 0, pre=it0)
        nc.sync.dma_start(at_sb[:], at_d.ap())
        nc.sync.dma_start(id_sb[:], id_d.ap())
        for u in range(1, 7):
            s123_unit(0, u)
        with tc.high_priority():
            norm_h(0, 0)
        s123_unit(0, 7)

        # ---- phase B: norm+transpose+M1 b0; prefetch b1's first inputs
        s123_open(1)
        with tc.high_priority():
            norm_h(0, 1)
        tpm1(0, help_dve=False)

        # ---- phase C: up/out b0 interleaved with s123 b1. The first
        # s123(b1) units are front-loaded so the DVE never stalls on a
        # cast whose up-matmuls aren't ready yet; vector casts ride only
        # on the early units, and norm(b1) is emitted right after the
        # last s123 unit so the DVE reaches it without queuing behind
        # late-unit casts.
        c_order = [0, 2, 4, 6, 1, 3, 5, 7]  # xh-major
        for u in range(5):
            s123_unit(1, u)
        for i in range(2):
            up_unit(0, c_order[i], n_dve=1)
            s123_unit(1, 5 + i)
        with tc.high_priority():
            norm_h(1, 0)
        up_unit(0, c_order[2], n_dve=1)
        s123_unit(1, 7)
        with tc.high_priority():
            norm_h(1, 1)
        # late b0 units: vector casts land in the window where the DVE
        # has finished norm(b1) and would otherwise idle behind ACT
        for i in range(3, 8):
            up_unit(0, c_order[i], n_dve=2)

        # ---- phase D: transpose+M1 b1, high priority so its ACT ops do
        # not queue behind up(0)'s remaining casts
        with tc.high_priority():
            tpm1(1, help_dve=True)

        # ---- phase E: up/out b1, xh-major so the first units wait only
        # on the sx<256 m1-cast halves
        for u in [0, 2, 4, 6, 1, 3, 5, 7]:
            up_unit(1, u, n_dve=2)

    nc.compile()
    return nc


def _host_prep(input, fc_weight3):
    inp = np.ascontiguousarray(input, dtype=np.float16)
    w0 = np.ascontiguousarray(fc_weight3[:, :, 0], dtype=np.float32)
    # w0: (N,16) with n=(b,gy,gx), k=(py,px)
    w0 = w0.reshape(B, 4, 32, G, PP, PP)          # b yt gy_l gx py px
    # per-partition row p=(gy_l,py), free=(yt, gx, px): contiguous per
    # batch; each weight duplicated x2 (dense inner pair for DVE 2x mode)
    w_arr = w0.transpose(0, 2, 4, 1, 3, 5).reshape(B, 128, 2048)
    w_arr = np.ascontiguousarray(
        np.repeat(w_arr, 2, axis=2).astype(np.float16))

    A = _interp_matrix_np(G, S)                   # (512,128)
    at = np.ascontiguousarray(A.T.astype(np.float16))  # (128,512)

    sel = np.zeros((128, 512), dtype=np.float16)
    p = np.arange(128)
    for j in range(4):
        sel[p, j * 128 + 32 * j + p // 4] = 1.0

    ident = np.eye(128, dtype=np.float16)
    return inp, w_arr, at, sel, ident


def kernel(input, fc_weight3):
    from concourse.bass_utils import run_bass_kernel_spmd

    if "nc" not in _CACHE:
        _CACHE["nc"] = _build_program()
    nc = _CACHE["nc"]

    inp, w_arr, at, sel, ident = _host_prep(input, fc_weight3)

    in_maps = []
    for c in range(NCORES):
        in_maps.append({
            "x": inp[c * BPC:(c + 1) * BPC],
            "w": w_arr[c * BPC:(c + 1) * BPC],
            "at": at,
            "sel": sel,
            "ident": ident,
        })
    res = run_bass_kernel_spmd(nc, in_maps, list(range(NCORES)))
    out = np.concatenate([r["y"] for r in res.results], axis=0)
    return out.reshape(-1, T).astype(np.float32)


# revision 24
# speedup vs baseline: 1.0057x; 1.0057x over previous
"""Trainium2 Bass kernel for nn_Fine_Change_Moment3.

Math (from the reference):
  - input (16,512,512,16) [b,y,x,t]; fc_weight3 (262144,16,6) per-patch 16x6.
  - Only channel 0 of the CAM survives (cam[:, 0]), so only
    fc_weight3[:, :, 0] matters (host-sliced).
  - Per 4x4 patch n=(b,gy,gx): cam0[t] = sum_k patch[k,t] * w[n,k]
  - v = (cam0 - min_t) / max_t(cam0 - min_t)
  - top[b,t] = v arranged (gy,gx); up = A @ top @ A^T with A the 128->512
    bilinear (align_corners) interp matrix; output (b*512*512, 16) f32.

Distribution: data-parallel over batch, 2 batches per core, 8 cores.

fp16 end-to-end on device: the host casts input and weights to fp16 and
casts the fp16 output back to f32, halving HBM traffic (the DMA roofline)
and doubling DVE/PE throughput (2e-2 tolerance; fp16 leaves ~1e-2 margin).

Engine queues are FIFO, so program order is the schedule. The two batches
are software-pipelined explicitly:

  A: load+weight+reduce (s123) b0          DVE-paced, in_b0 streams
  B: norm+transpose+M1 b0; prefetch in_b1
  C: up/out b0 INTERLEAVED with s123 b1    ACT+PE run b0's tail while
                                           DVE consumes b1's input;
                                           out_b0 + in_b1 share DMA
  D: norm+transpose+M1 b1
  E: up/out b1                             copies split DVE/ACT (nothing
                                           queues behind them)

Stage detail:
  s123: DVE multiply by per-patch weights (weights host-duplicated x2 so
        the t-broadcast keeps a dense step-1 inner pair -> 2x DVE mode),
        one DVE add pass folding px pairs into p2[(px2,gx,t)] (px2
        outermost so matmul rhs chunks stay dense), then doubled
        0/1-selection matmuls reduce py AND the final px2 pair via PSUM
        accumulation -> cam[gy][(gx,t)] fp32.
  norm: v = cam - min only (computed per cam half: the gx<64 selection
        groups stop one input unit earlier, so that half of the norm
        runs before the last unit and only the gx>=64 half sits on the
        critical chain). The 1/max scale is applied AFTER the per-t
        transposes (elementwise, commutes), so the transposes depend
        only on the subtract while max/recip overlap them; rx rides the
        transpose pass as a 17th 128x128 tile and scales topT in place.
        Emitted at high priority so the scheduler does not stretch the
        chain by interleaving b1 multiplies into it.
  M1:   PE per-t transposes then M1_t = topT_t^T @ A^T; casts into
        m1i[gy][(sx,t)] are sx-halved so the xh=0 up-units (emitted
        xh-major) wait on only half of them. For b1 the whole stage is
        emitted at high priority so its ACT ops do not queue behind
        up(0)'s remaining casts (all of E depends on m1i(1)).
  up:   PE (A^T chunk)^T @ m1i chunk -> PSUM, copies cast to fp16
        staging (split across ACT and DVE by phase load), contiguous
        DMA out (the final unit's DMA is split in half to start
        earlier). On the last batch the idle cam PSUM pool is borrowed
        as a third ring slot so the PE runs ahead of the copies.

Engine facts this schedule is built around (measured):
  - DVE ~1.04ns/elem (2x for fp16 SBUF-to-SBUF with packed inner pair);
    f32/PSUM reads are 1x. ACT ~0.9-1.1ns/elem regardless of dtype.
    gpsimd cannot access PSUM at all, so all PSUM->SBUF casts must go
    through ACT or DVE - they are the binding resource of the back half.
  - TRN2 matmul may only emit f32 to PSUM, a matmul dst cannot cross a
    2KB PSUM bank, and the PE runs ~0.83ns/col (PSTATE_MID) unless
    continuously busy for 3us+ (never happens here).
  - DMA sustains ~337GB/s; in+out+weights ~36MB/core => ~106us floor.
"""

import numpy as np

B, S, T, PP = 16, 512, 16, 4
G = S // PP          # 128 patch grid
NCORES = 8
BPC = B // NCORES    # 2 batches per core

_CACHE = {}


def _interp_matrix_np(n_in, n_out):
    # mirrors the reference's align_corners=True bilinear matrix
    coords = np.arange(n_out, dtype=np.float32) * ((n_in - 1) / (n_out - 1))
    i0 = np.clip(np.floor(coords).astype(np.int64), 0, n_in - 2)
    w = coords - i0.astype(np.float32)
    A = np.zeros((n_out, n_in), dtype=np.float32)
    rows = np.arange(n_out)
    np.add.at(A, (rows, i0), 1.0 - w)
    np.add.at(A, (rows, i0 + 1), w)
    return A  # (n_out, n_in)


def _build_program():
    from contextlib import ExitStack
    import concourse.bacc as bacc
    import concourse.tile as tile
    import concourse.mybir as mybir

    f32 = mybir.dt.float32
    f16 = mybir.dt.float16
    Alu = mybir.AluOpType
    Ax = mybir.AxisListType

    nc = bacc.Bacc("TRN2", target_bir_lowering=False, debug=False,
                   num_devices=NCORES)

    x_d = nc.dram_tensor("x", [BPC, S, S, T], f16, kind="ExternalInput")
    w_d = nc.dram_tensor("w", [BPC, 128, 4096], f16, kind="ExternalInput")
    at_d = nc.dram_tensor("at", [128, 512], f16, kind="ExternalInput")
    sel_d = nc.dram_tensor("sel", [128, 512], f16, kind="ExternalInput")
    id_d = nc.dram_tensor("ident", [128, 128], f16, kind="ExternalInput")
    y_d = nc.dram_tensor("y", [BPC, S, S, T], f16, kind="ExternalOutput")

    # input view: [b][yt][xh][y_row=128][(x256 t16)=4096]
    x_v = x_d.ap().rearrange("b (yt p) (xh xx) t -> b yt xh p (xx t)",
                             p=128, xh=2)
    # output view: [b][syc][xh][sy=128][(xx256 t16)=4096]
    y_v = y_d.ap().rearrange("b (syc sy) (xh xx) t -> b syc xh sy (xx t)",
                             syc=4, xh=2)

    with tile.TileContext(nc) as tc, ExitStack() as ctx:
        consts = ctx.enter_context(tc.tile_pool(name="consts", bufs=1))
        pin = ctx.enter_context(tc.tile_pool(name="pin", bufs=10))
        pw = ctx.enter_context(tc.tile_pool(name="pw", bufs=8))
        pp2 = ctx.enter_context(tc.tile_pool(name="pp2", bufs=4))
        pv = ctx.enter_context(tc.tile_pool(name="pv", bufs=1))
        ptop = ctx.enter_context(tc.tile_pool(name="ptop", bufs=1))
        pm1 = ctx.enter_context(tc.tile_pool(name="pm1", bufs=1))
        pst = ctx.enter_context(tc.tile_pool(name="pst", bufs=5))
        # PSUM: cam+transpose share a 4-bank pool; m1/up share a
        # 2x 2-bank pool -> static total 8 banks.
        ppsc = ctx.enter_context(tc.tile_pool(name="ppsc", bufs=1,
                                              space="PSUM"))
        ppsmu = ctx.enter_context(tc.tile_pool(name="ppsmu", bufs=2,
                                               space="PSUM"))

        at_sb = consts.tile([128, 512], f16)
        sel_sb = consts.tile([128, 512], f16)
        id_sb = consts.tile([128, 128], f16)

        state = {}

        def s123_open(b):
            # weight prefetch in per-yt chunks (first multiply only waits
            # on chunk 0); cam PSUM is allocated lazily at its first
            # matmul so the ppsc ring order stays cam(b)/tp(b)
            wv4 = w_d.ap().rearrange("b p (yt c) -> b yt p c", yt=4)
            wts = []
            for yt in range(4):
                wt = pw.tile([128, 1024], f16, tag="w", name="w_sb")
                nc.sync.dma_start(wt[:], wv4[b, yt])
                wts.append(wt)
            state[b] = {"cam_ps": None, "w_sb": wts, "p2": None}

        def s123_unit(b, u, pre=None, fold_pool=False):
            # one (yt, xh) input unit: DMA + multiply + add tree (+ cam
            # matmuls when the yt pair completes)
            yt, xh = u // 2, u % 2
            st = state[b]
            w_sb = st["w_sb"]
            if xh == 0:
                st["p2"] = pp2.tile([128, 4096], f16, tag="p2", name="p2")
            p2 = st["p2"]
            if pre is None:
                it = pin.tile([128, 4096], f16, tag="in")
                nc.sync.dma_start(it[:], x_v[b, yt, xh])
            else:
                it = pre
            itv = it[:].rearrange("p (x tp two) -> p x tp two",
                                  tp=T // 2, two=2)
            wv = (w_sb[yt][:, xh * 512:(xh + 1) * 512]
                  .rearrange("p (x two) -> p x two", two=2)
                  .unsqueeze(2)
                  .broadcast_to([128, 256, T // 2, 2]))
            if pre is not None:
                # unit 0: the tile's DMA arrives in two halves; a halved
                # multiply starts ~1.5us earlier
                for hx in range(2):
                    nc.vector.tensor_tensor(itv[:, hx * 128:(hx + 1) * 128],
                                            itv[:, hx * 128:(hx + 1) * 128],
                                            wv[:, hx * 128:(hx + 1) * 128],
                                            op=Alu.mult)
            else:
                nc.vector.tensor_tensor(itv, itv, wv, op=Alu.mult)
            # single add pass folds px {0,2}+{1,3} -> p2[(px2, gx, t)]
            # (px2 outermost so the doubled selection matmuls read dense
            # 512-chunks); the final px2 pair is summed by PSUM
            # accumulation in the selection matmuls
            pr = it[:].rearrange("p (gx pxp px2 t) -> p px2 gx t pxp",
                                 pxp=2, px2=2, t=T)
            rout = (p2[:].rearrange("p (px2 gx t) -> p px2 gx t",
                                    px2=2, t=T)[:, :, xh * 64:(xh + 1) * 64])
            # gpsimd folds run ~4x slower but fully concurrent with the
            # DVE mult stream; giving it the first half of b1's folds
            # shortens the DVE spine in phase C
            feng = nc.gpsimd if fold_pool else nc.vector
            feng.tensor_tensor(rout, pr[:, :, :, :, 0],
                               pr[:, :, :, :, 1], op=Alu.add)
            # selection matmuls for THIS xh-half (fc 2xh, 2xh+1): they
            # read only this unit's fold output, so the gx<64 cam half
            # finalizes at unit 6 and norm_h(b,0) can run before unit 7
            if st["cam_ps"] is None:
                st["cam_ps"] = ppsc.tile([128, 2048], f32, tag="cam",
                                     name="cam_ps")
            cam_ps = st["cam_ps"]
            for fcl in range(2):
                fc = 2 * xh + fcl
                for px2 in range(2):
                    nc.tensor.matmul(
                        cam_ps[:, fc * 512:(fc + 1) * 512],
                        lhsT=sel_sb[:, yt * 128:(yt + 1) * 128],
                        rhs=p2[:, px2 * 2048 + fc * 512:
                               px2 * 2048 + (fc + 1) * 512],
                        start=(yt == 0 and px2 == 0),
                        stop=(yt == 3 and px2 == 1),
                    )

        def norm_h(b, h):
            # v = cam - min only; the 1/max scaling is applied AFTER the
            # per-t transposes (it commutes elementwise), so the
            # transposes depend only on the subtract and the max/recip
            # run concurrently with the PE transposes. Split per cam
            # half: the selection-matmul groups for gx 0..63 stop one
            # input unit earlier than gx 64..127, so half the norm runs
            # before the last unit's mult/fold and only the h=1 half
            # sits on the critical chain.
            st = state[b]
            cam_ps = st["cam_ps"]
            if h == 0:
                st["v"] = pv.tile([128, 2048], f16, tag="v", name="v")
                st["mn"] = pv.tile([128, 128], f16, tag="mn", name="mn")
                st["mx"] = pv.tile([128, 128], f32, tag="mx", name="mx")
                st["rx"] = pv.tile([128, 128], f16, tag="rx", name="rx")
            v, mn, mx, rx = st["v"], st["mn"], st["mx"], st["rx"]
            cam3 = (cam_ps[:, h * 1024:(h + 1) * 1024]
                    .rearrange("p (gx t) -> p gx t", t=T))
            v3 = (v[:, h * 1024:(h + 1) * 1024]
                  .rearrange("p (gx t) -> p gx t", t=T))
            mnh = mn[:, h * 64:(h + 1) * 64]
            mxh = mx[:, h * 64:(h + 1) * 64]
            rxh = rx[:, h * 64:(h + 1) * 64]
            nc.vector.tensor_reduce(mnh, cam3, axis=Ax.X, op=Alu.min)
            mnb = mnh.unsqueeze(2).broadcast_to([128, 64, T])
            nc.vector.tensor_tensor(v3, cam3, mnb, op=Alu.subtract)
            nc.vector.tensor_reduce(mxh, v3, axis=Ax.X, op=Alu.max)
            with nc.allow_low_precision(reason="1/max in f16: |err|<1e-3 "
                                        "vs the 2e-2 gate; everything "
                                        "downstream is f16 anyway"):
                nc.vector.reciprocal(rxh, mxh)

        def tpm1(b, help_dve):
            st = state[b]
            v = st["v"]
            rx = st["rx"]

            # per-t 128x128 transposes -> topT[gx][(t,gy)]; these wait
            # only on the norm subtract (v is unscaled). rx rides along
            # as a 17th transpose; the scale lands on topT afterwards.
            topT = ptop.tile([128, 2048], f16, tag="top")
            rxT = ptop.tile([128, 128], f16, tag="rxT")
            vt = v[:].rearrange("p (gx t) -> p t gx", t=T)
            # 2176 wide: cols 2048+ hold the rx transpose (the tag's
            # slot is sized by the f32 cam tile anyway)
            tp_ps = ppsc.tile([128, 2176], f16, tag="cam")
            for t in range(T):
                nc.tensor.transpose(tp_ps[:, t * 128:(t + 1) * 128],
                                    vt[:, t, :], id_sb[:])
            nc.tensor.transpose(tp_ps[:, 2048:2176], rx[:], id_sb[:])
            if help_dve:
                # b1: the DVE is idle here while ACT still drains up(0)
                # casts; fp16 PSUM->SBUF on the DVE is 2x-capable
                nc.vector.tensor_copy(topT[:], tp_ps[:, 0:2048])
                nc.scalar.copy(rxT[:], tp_ps[:, 2048:2176])
            else:
                nc.scalar.copy(topT[:], tp_ps[:, 0:2048])
                nc.vector.tensor_copy(rxT[:], tp_ps[:, 2048:2176])
            # apply the 1/max scale to the transposed plane: topT[gx,t,gy]
            # *= rxT[gx,gy] (broadcast over t; gy inner stride-1 pair)
            topTv = topT[:].rearrange("p (t gy) -> p t gy", t=T)
            rxb = rxT[:].unsqueeze(1).broadcast_to([128, T, 128])
            nc.vector.tensor_tensor(topTv, topTv, rxb, op=Alu.mult)

            # M1 sx-half-major: ALL sx<256 matmuls+casts run first, so
            # the xh=0 up-units (emitted xh-major) launch after only half
            # the M1 stream; the sx>=256 half completes while they run
            m1i = pm1.tile([128, 8192], f16, tag="m1i")
            m1iv = m1i[:].rearrange("p (sx t) -> p sx t", t=T)
            kk = 0
            for sh in range(2):
                for tq in range(8):
                    # on the last batch, borrow the idle cam pool as a
                    # third ring slot so the PE runs ahead of the copies
                    if help_dve and kk % 3 == 2:
                        m1_ps = ppsc.tile([128, 512], f32, tag="cam",
                                          name="m1_ps")
                    else:
                        m1_ps = ppsmu.tile([128, 512], f32, tag="mu",
                                           name="m1_ps")
                    for tl in range(2):
                        t = tq * 2 + tl
                        nc.tensor.matmul(
                            m1_ps[:, tl * 256:(tl + 1) * 256],
                            lhsT=topT[:, t * 128:(t + 1) * 128],
                            rhs=at_sb[:, sh * 256:(sh + 1) * 256],
                            start=True, stop=True,
                        )
                    csrc = m1_ps[:].rearrange("p (tl sxh) -> p sxh tl",
                                              tl=2)
                    cdst = m1iv[:, sh * 256:(sh + 1) * 256,
                                tq * 2:(tq + 1) * 2]
                    if help_dve and kk % 2 == 1:
                        nc.vector.tensor_copy(cdst, csrc)
                    else:
                        nc.scalar.copy(cdst, csrc)
                    kk += 1
            st["m1i"] = m1i

        up_ctr = [0]

        def up_unit(b, u, n_dve=0):
            # one (syc, xh) output unit: 4x (2 matmuls + copy), DMA out
            syc, xh = u // 2, u % 2
            m1i = state[b]["m1i"]
            stg = pst.tile([128, 4096], f16, tag="stg")
            for sxg in range(4):
                gi = up_ctr[0]
                up_ctr[0] += 1
                if b == BPC - 1 and gi % 3 == 2:
                    up_ps = ppsc.tile([128, 1024], f32, tag="cam",
                                      name="up_ps")
                else:
                    up_ps = ppsmu.tile([128, 1024], f32, tag="mu",
                                       name="up_ps")
                for sxl in range(2):
                    sxblk = (xh * 4 + sxg) * 2 + sxl
                    nc.tensor.matmul(
                        up_ps[:, sxl * 512:(sxl + 1) * 512],
                        lhsT=at_sb[:, syc * 128:(syc + 1) * 128],
                        rhs=m1i[:, sxblk * 512:(sxblk + 1) * 512],
                        start=True, stop=True,
                    )
                dst = stg[:, sxg * 1024:(sxg + 1) * 1024]
                if (n_dve >= 2 and sxg % 2 == 1) or (n_dve == 1 and sxg == 3):
                    nc.vector.tensor_copy(dst, up_ps[:])
                else:
                    nc.scalar.copy(dst, up_ps[:])
                if b == BPC - 1 and u in (5, 7) and sxg == 1:
                    nc.sync.dma_start(y_v[b, syc, xh][:, 0:2048],
                                      stg[:, 0:2048])
            if b == BPC - 1 and u in (5, 7):
                nc.sync.dma_start(y_v[b, syc, xh][:, 2048:4096],
                                  stg[:, 2048:4096])
            else:
                nc.sync.dma_start(y_v[b, syc, xh], stg[:])

        # ---- phase A: s123(b0); the first input tile's DMA is issued
        # before the weight chunks; consts ride behind the first DMAs
        it0 = pin.tile([128, 4096], f16, tag="in", name="it0")
        s123_open(0)
        nc.sync.dma_start(it0[:, 0:2048], x_v[0, 0, 0][:, 0:2048])
        nc.sync.dma_start(it0[:, 2048:4096], x_v[0, 0, 0][:, 2048:4096])
        # sel_sb must be loaded before unit 0 (its selection matmuls now
        # run every unit, and emission order defines the dependency)
        nc.sync.dma_start(sel_sb[:], sel_d.ap())
        s123_unit(0, 0, pre=it0)
        nc.sync.dma_start(at_sb[:], at_d.ap())
        nc.sync.dma_start(id_sb[:], id_d.ap())
        for u in range(1, 7):
            s123_unit(0, u)
        with tc.high_priority():
            norm_h(0, 0)
        s123_unit(0, 7)

        # ---- phase B: norm+transpose+M1 b0; prefetch b1's first inputs
        s123_open(1)
        with tc.high_priority():
            norm_h(0, 1)
        tpm1(0, help_dve=False)

        # ---- phase C: up/out b0 interleaved with s123 b1. The first
        # s123(b1) units are front-loaded so the DVE never stalls on a
        # cast whose up-matmuls aren't ready yet; vector casts ride only
        # on the early units, and norm(b1) is emitted right after the
        # last s123 unit so the DVE reaches it without queuing behind
        # late-unit casts.
        c_order = [0, 2, 4, 6, 1, 3, 5, 7]  # xh-major
        for u in range(5):
            s123_unit(1, u)
        for i in range(2):
            up_unit(0, c_order[i], n_dve=1)
            s123_unit(1, 5 + i)
        with tc.high_priority():
            norm_h(1, 0)
        up_unit(0, c_order[2], n_dve=1)
        s123_unit(1, 7)
        with tc.high_priority():
            norm_h(1, 1)
        # late b0 units: vector casts land in the window where the DVE
        # has finished norm(b1) and would otherwise idle behind ACT
        for i in range(3, 8):
            up_unit(0, c_order[i], n_dve=2)

        # ---- phase D: transpose+M1 b1, high priority so its ACT ops do
        # not queue behind up(0)'s remaining casts
        with tc.high_priority():
            tpm1(1, help_dve=True)

        # ---- phase E: up/out b1, xh-major so the first units wait only
        # on the sx<256 m1-cast halves
        for u in [0, 2, 4, 6, 1, 3, 5, 7]:
            up_unit(1, u, n_dve=2)

    nc.compile()
    return nc


def _host_prep(input, fc_weight3):
    inp = np.ascontiguousarray(input, dtype=np.float16)
    w0 = np.ascontiguousarray(fc_weight3[:, :, 0], dtype=np.float32)
    # w0: (N,16) with n=(b,gy,gx), k=(py,px)
    w0 = w0.reshape(B, 4, 32, G, PP, PP)          # b yt gy_l gx py px
    # per-partition row p=(gy_l,py), free=(yt, gx, px): contiguous per
    # batch; each weight duplicated x2 (dense inner pair for DVE 2x mode)
    w_arr = w0.transpose(0, 2, 4, 1, 3, 5).reshape(B, 128, 2048)
    w_arr = np.ascontiguousarray(
        np.repeat(w_arr, 2, axis=2).astype(np.float16))

    A = _interp_matrix_np(G, S)                   # (512,128)
    at = np.ascontiguousarray(A.T.astype(np.float16))  # (128,512)

    sel = np.zeros((128, 512), dtype=np.float16)
    p = np.arange(128)
    for j in range(4):
        sel[p, j * 128 + 32 * j + p // 4] = 1.0

    ident = np.eye(128, dtype=np.float16)
    return inp, w_arr, at, sel, ident


def kernel(input, fc_weight3):
    from concourse.bass_utils import run_bass_kernel_spmd

    if "nc" not in _CACHE:
        _CACHE["nc"] = _build_program()
    nc = _CACHE["nc"]

    inp, w_arr, at, sel, ident = _host_prep(input, fc_weight3)

    in_maps = []
    for c in range(NCORES):
        in_maps.append({
            "x": inp[c * BPC:(c + 1) * BPC],
            "w": w_arr[c * BPC:(c + 1) * BPC],
            "at": at,
            "sel": sel,
            "ident": ident,
        })
    res = run_bass_kernel_spmd(nc, in_maps, list(range(NCORES)))
    out = np.concatenate([r["y"] for r in res.results], axis=0)
    return out.reshape(-1, T).astype(np.float32)
